# revision 20
# baseline (speedup 1.0000x reference)
"""Trainium2 Bass kernel for a conditional GRU decoder.

Model (per reference):
  h0 = [z, x_cond] @ W_lh.T + b_lh
  x0 = 0
  for t in 0..127:
      hn = GRUCell(x_t, h_t);  logits_t = hn @ W_out.T + b_out;  x_{t+1} = hn
  out = (B, 128, 64)

Because x_{t+1} == h_{t+1} for t >= 1, the two GRU matmuls fuse into one
(B,R) @ (R,4R) matmul with W_fused rows [Wi_r+Wh_r; Wi_z+Wh_z; Wi_n; Wh_n].

Sharding: data-parallel over batch, B=2048 -> 8 cores x 256. All weights
replicated. On-chip layout is transposed (feature dim on partitions, batch on
the free dim) so gate biases are per-partition scalars and the recurrent
matmuls keep weights stationary:
    gatesT[4R, b] = W_fused @ hT   via  matmul(out, lhsT=W_fused.T, rhs=hT)

Engine split per step (chunk c = feature rows c*128..c*128+127):
  PE:   16 gate MMs (N=256 f16) + logits (2 MMs + K=1 ones-MM injecting
        b_out into PSUM, so logits DMA straight from PSUM with no ACT pass)
        + one K=2 selector bias-MM injecting b_z into the gz bank (so the
        u-sigmoid is a single [128,512] ACT instr).
  ACT:  r = sigmoid per chunk (bias in ACT), u = sigmoid [128,512],
        n = tanh per chunk (bias pre-added by the pre-STT).
  DVE:  t2_c=(b_hn+ghn)*r (STT), pre_c=(b_in+gin)+t2 (STT),
        p1n_c=(u-1)*n (STT f16), h'_0=p2_0-p1n_0 (chain chunk).
  Pool: p2_c=u*h, h'_1 (off-chain chunk, staggered for the k1 MMs).

All matmul operands f16 (measured end-to-end rel err ~1e-3 scale); gate
accumulation and logits stay f32 in PSUM.
"""

import numpy as np

import concourse.bass as bass
import concourse.tile as tile
from concourse import bacc, mybir
from concourse.bass_utils import run_bass_kernel_spmd

F32 = mybir.dt.float32
F16 = mybir.dt.float16
ACT = mybir.ActivationFunctionType
ALU = mybir.AluOpType

B = 2048
HID = 256
COND = 128
NCH = 64
MAXLEN = 128
R = 256
NCORES = 8
BC = B // NCORES  # 256 per-core batch
KT = R // 128     # 2 k-tiles over R
ZC = HID + COND   # 384
ZKT = ZC // 128   # 3 k-tiles over hid+cond


def _build(maxlen=MAXLEN, dbg=False):
    nc = bacc.Bacc("TRN2", target_bir_lowering=False, debug=False)

    # ---- DRAM I/O (per-core shapes) ----
    d_zct = nc.dram_tensor("zct", [ZC, BC], F16, kind="ExternalInput")
    d_wf = nc.dram_tensor("wft", [R, 4 * R], F16, kind="ExternalInput")
    d_whh = nc.dram_tensor("whht", [R, 3 * R], F16, kind="ExternalInput")
    d_wlh = nc.dram_tensor("wlht", [ZC, R], F16, kind="ExternalInput")
    d_wout = nc.dram_tensor("woutt", [R, NCH], F16, kind="ExternalInput")
    # per-partition bias columns: 0,1=b_r  2,3=b_hn  4,5=b_in (per chunk)
    d_bias = nc.dram_tensor("biases", [128, 6], F32, kind="ExternalInput")
    # selector-MM operands
    d_bzl = nc.dram_tensor("bzl", [KT, 128], F16, kind="ExternalInput")
    d_blhl = nc.dram_tensor("blhl", [KT, 128], F16, kind="ExternalInput")
    d_bout = nc.dram_tensor("bout", [NCH, 1], F32, kind="ExternalInput")
    d_sel2 = nc.dram_tensor("sel2", [KT, KT * BC], F16, kind="ExternalInput")
    d_out = nc.dram_tensor("out", [maxlen, NCH, BC], F32, kind="ExternalOutput")
    d_ka = nc.dram_tensor("kadbg", [128, BC], F32, kind="ExternalOutput")
    d_dbg = {}
    if dbg:
        for nm in ("h0", "r", "u", "t2", "nt", "p2", "p1n", "hs", "gr",
                   "ghn"):
            d_dbg[nm] = nc.dram_tensor("dbg_" + nm, [128, KT, BC], F32,
                                       kind="ExternalOutput")

    with tile.TileContext(nc) as tc:
        with (
            tc.tile_pool(name="const", bufs=1) as const,
            tc.tile_pool(name="state", bufs=1) as state,
            tc.tile_pool(name="ew", bufs=2) as ew,
            tc.tile_pool(name="pgr", bufs=1, space="PSUM") as pgr,
            tc.tile_pool(name="pghn", bufs=1, space="PSUM") as pghn,
            tc.tile_pool(name="pgin", bufs=1, space="PSUM") as pgin,
            tc.tile_pool(name="pgz", bufs=1, space="PSUM") as pgz,
            tc.tile_pool(name="pl", bufs=2, space="PSUM") as pl,
            tc.tile_pool(name="ph0", bufs=1, space="PSUM") as ph0,
        ):
            # ---- load constants ----
            wf = const.tile([128, KT, 4 * R], F16)
            nc.sync.dma_start(wf, d_wf[:].rearrange("(k p) m -> p k m", p=128))
            whh = const.tile([128, KT, 3 * R], F16)
            nc.sync.dma_start(whh, d_whh[:].rearrange("(k p) m -> p k m", p=128))
            wlh = const.tile([128, ZKT, R], F16)
            nc.sync.dma_start(wlh, d_wlh[:].rearrange("(k p) m -> p k m", p=128))
            wout = const.tile([128, KT, NCH], F16)
            nc.sync.dma_start(wout, d_wout[:].rearrange("(k p) m -> p k m", p=128))
            zct = const.tile([128, ZKT, BC], F16)
            nc.sync.dma_start(zct, d_zct[:].rearrange("(k p) m -> p k m", p=128))
            bia = const.tile([128, 6], F32)
            nc.sync.dma_start(bia, d_bias[:])
            bzl = const.tile([KT, 128], F16)
            nc.sync.dma_start(bzl, d_bzl[:])
            blhl = const.tile([KT, 128], F16)
            nc.sync.dma_start(blhl, d_blhl[:])
            boutc = const.tile([NCH, 1], F32)
            nc.sync.dma_start(boutc, d_bout[:])
            sel2 = const.tile([KT, KT * BC], F16)
            nc.sync.dma_start(sel2, d_sel2[:])

            def bcol(i):
                return bia[:, i : i + 1]

            h = state.tile([128, KT, BC], F16)  # hT, chunk c = rows c*128..

            # PE keepalive: dummy accumulating matmuls on chain tiles keep
            # the HAM clock-gate at full rate through the per-step EW gap.
            ka = ph0.tile([128, BC], F32, tag="ka")
            ka_started = [False]

            def keepalive_on(rhs):
                nc.tensor.matmul(ka, wf[:, 0, 0:128], rhs,
                                 start=(not ka_started[0]), stop=False,
                                 skip_group_check=True)
                ka_started[0] = True

            # ---- HAM warmup: ~4.5us of back-to-back MMs so the SHORT
            # window fires and the PE clock un-throttles to 2.4 GHz; the
            # per-step keepalives then keep every MID window non-idle.
            for _ in range(22):
                keepalive_on(zct[:, 0, :])

            # ---- h0 = W_lh @ zcT + b_lh (selector bias-MM + 3 k-tiles) ----
            ph = ph0.tile([128, KT, BC], F32, tag="h0")
            nc.tensor.matmul(ph[:], blhl[:], sel2[:],
                             start=True, stop=False, skip_group_check=True)
            for m in range(KT):
                for k in range(ZKT):
                    nc.tensor.matmul(ph[:, m, :], wlh[:, k, bass.ts(m, 128)],
                                     zct[:, k, :], start=False,
                                     stop=(m == KT - 1 and k == ZKT - 1),
                                     skip_group_check=True)
            nc.scalar.activation(h[:],
                                 ph[:], ACT.Identity,
                                 bias=0.0)
            dbg_f32 = {}
            if dbg:
                for nm in d_dbg:
                    dbg_f32[nm] = const.tile([128, KT, BC], F32,
                                             name="dbgt_" + nm)
                nc.vector.tensor_copy(dbg_f32["h0"], h[:])
                nc.sync.dma_start(d_dbg["h0"][:], dbg_f32["h0"])

            # pending logits emitters from the previous step
            pending = []

            def flush_logits():
                for fn in pending:
                    fn()
                pending.clear()

            def emit_step(t, first):
                if first:
                    # x=0: gates come from W_hh only (pytorch order r,z,n)
                    w, offs = whh, {"r": 0, "z": R, "hn": 2 * R}
                else:
                    w, offs = wf, {"r": 0, "z": R, "in": 2 * R, "hn": 3 * R}

                gr = pgr.tile([128, KT, BC], F32, tag="gr")
                ghn = pghn.tile([128, KT, BC], F32, tag="ghn")
                gin = pgin.tile([128, KT, BC], F32, tag="gin")
                gz = pgz.tile([128, KT, BC], F32, tag="gz")

                def gate_mm(dst, name, m, k, start, stop):
                    nc.tensor.matmul(
                        dst[:, m, :], w[:, k, bass.ds(offs[name] + m * 128, 128)],
                        h[:, k, :], start=start, stop=stop,
                        skip_group_check=True)

                # ---- PE burst ----
                # bias-MMs first: no h dependency, run during prev EW tail
                nc.tensor.matmul(gz[:], bzl[:], sel2[:],
                                 start=True, stop=False, skip_group_check=True)
                # One start=True per bank (PSUM pending-zero is
                # bank-granular): first MM starts, last MM stops.
                names = ("r", "hn") if first else ("r", "hn", "in")
                # k0 round (waits h chunk0 only), r first (chain head)
                for name in names + ("z",):
                    for m in range(KT):
                        gate_mm({"r": gr, "hn": ghn, "in": gin, "z": gz}[name],
                                name, m, 0,
                                start=(name in ("r", "hn", "in") and m == 0),
                                stop=False)
                # pending logits (consume full h(t-1), same dep as k1 round)
                flush_logits()
                # k1 round (waits h chunk1)
                for name in names + ("z",):
                    for m in range(KT):
                        gate_mm({"r": gr, "hn": ghn, "in": gin, "z": gz}[name],
                                name, m, 1, start=False,
                                stop=(name in ("r", "hn", "z") and m == KT - 1))
                # t2 -> gin bank via identity-weight MMs (replaces pre-STT)
                # (emitted after the elementwise t2 below via a closure list)

                if dbg and t == 0:
                    nc.vector.tensor_copy(dbg_f32["gr"], gr[:])
                    nc.sync.dma_start(d_dbg["gr"][:], dbg_f32["gr"])
                    nc.vector.tensor_copy(dbg_f32["ghn"], ghn[:])
                    nc.sync.dma_start(d_dbg["ghn"][:], dbg_f32["ghn"])
                # ---- ACT: r per chunk (chain head), u merged ----
                r = ew.tile([128, KT, BC], F16, tag="r")
                with tc.high_priority(64):
                    for m in range(KT):
                        nc.scalar.activation(r[:, m, :], gr[:, m, :],
                                             ACT.Sigmoid, bias=bcol(0 + m))
                u = ew.tile([128, KT, BC], F16, tag="u")
                nc.scalar.activation(u[:],
                                     gz[:], ACT.Sigmoid,
                                     bias=0.0)

                # ---- DVE: t2 = (b_hn + ghn) * r, pre = (b_in + gin) + t2
                t2 = ew.tile([128, KT, BC], F16, tag="t2")
                pre = ew.tile([128, KT, BC], F16, tag="pre")
                nt = ew.tile([128, KT, BC], F16, tag="nt")
                for m in range(KT):
                    nc.vector.scalar_tensor_tensor(
                        t2[:, m, :], ghn[:, m, :], bcol(2 + m), r[:, m, :],
                        op0=ALU.add, op1=ALU.mult)
                    if first:
                        nc.vector.tensor_scalar_add(pre[:, m, :], t2[:, m, :],
                                                    bcol(4 + m))
                    else:
                        nc.vector.scalar_tensor_tensor(
                            pre[:, m, :], gin[:, m, :], bcol(4 + m),
                            t2[:, m, :], op0=ALU.add, op1=ALU.add)
                    nc.scalar.activation(nt[:, m, :], pre[:, m, :], ACT.Tanh,
                                         bias=0.0)

                # ---- DVE chain: (after tanh) p1n, h' ----
                p1n = ew.tile([128, KT, BC], F16, tag="p1n")
                p2 = ew.tile([128, KT, BC], F16, tag="p2")
                # p2 = u*h on Pool (off-chain; reads old h)
                for m in range(KT):
                    nc.gpsimd.tensor_mul(p2[:, m, :], u[:, m, :], h[:, m, :])
                # p1n = (u-1)*n ; h' = p2 - p1n = u*h + (1-u)*n
                for m in range(KT):
                    nc.vector.scalar_tensor_tensor(
                        p1n[:, m, :], u[:, m, :], -1.0, nt[:, m, :],
                        op0=ALU.add, op1=ALU.mult)
                    nc.vector.tensor_sub(h[:, m, :], p2[:, m, :],
                                         p1n[:, m, :])
                if dbg and t == 0:
                    for nm, tl in (("r", r), ("u", u), ("t2", t2),
                                   ("nt", nt), ("p2", p2),
                                   ("p1n", p1n), ("hs", h)):
                        nc.vector.tensor_copy(dbg_f32[nm], tl[:])
                        nc.sync.dma_start(d_dbg[nm][:], dbg_f32[nm])

                # keepalives spread through the EW tail
                keepalive_on(r[:, 0, :])
                keepalive_on(t2[:, 1, :])
                keepalive_on(nt[:, 0, :])
                keepalive_on(p1n[:, 1, :])

                # ---- logits (deferred into next burst) ----
                lp = pl.tile([NCH, BC], F32, tag="lp")

                def do_logits(lp=lp, t=t):
                    for k in range(KT):
                        nc.tensor.matmul(lp, wout[:, k, :], h[:, k, :],
                                         start=(k == 0), stop=(k == KT - 1),
                                         skip_group_check=True)
                    ls = ew.tile([NCH, BC], F32, tag="ls")
                    nc.scalar.activation(ls, lp, ACT.Identity,
                                         bias=boutc[:, 0:1])
                    nc.sync.dma_start(d_out[t], ls)
                pending.append(do_logits)

            emit_step(0, first=True)
            for t in range(1, maxlen):
                emit_step(t, first=False)
            flush_logits()
            kcopy = ew.tile([128, BC], F32, tag="kcopy")
            nc.scalar.activation(kcopy, ka, ACT.Identity, bias=0.0)
            nc.sync.dma_start(d_ka[:], kcopy)

    nc.compile()
    return nc


_CACHE = {}
_LAST_IN_MAPS = None


def kernel(z, x_cond, W_lh, b_lh, W_ih, W_hh, b_ih, b_hh, W_out, b_out):
    z = np.asarray(z, np.float32)
    x_cond = np.asarray(x_cond, np.float32)
    W_lh = np.asarray(W_lh, np.float32)
    b_lh = np.asarray(b_lh, np.float32)
    W_ih = np.asarray(W_ih, np.float32)
    W_hh = np.asarray(W_hh, np.float32)
    b_ih = np.asarray(b_ih, np.float32)
    b_hh = np.asarray(b_hh, np.float32)
    W_out = np.asarray(W_out, np.float32)
    b_out = np.asarray(b_out, np.float32)

    # fused recurrent weight: rows [Wi_r+Wh_r; Wi_z+Wh_z; Wi_n; Wh_n]
    Wf = np.concatenate(
        [W_ih[:R] + W_hh[:R], W_ih[R : 2 * R] + W_hh[R : 2 * R],
         W_ih[2 * R :], W_hh[2 * R :]], axis=0)
    b_r = b_ih[:R] + b_hh[:R]
    b_z = b_ih[R : 2 * R] + b_hh[R : 2 * R]
    b_in = b_ih[2 * R :]
    b_hn = b_hh[2 * R :]

    def pcols(v):  # (R,) -> (128, KT) per-partition columns
        return np.ascontiguousarray(v.reshape(KT, 128).T)

    biases = np.ascontiguousarray(
        np.concatenate([pcols(b_r), pcols(b_hn), pcols(b_in)],
                       axis=1))  # (128, 6)

    f16 = np.float16
    wft = np.ascontiguousarray(Wf.T, dtype=f16)            # (R, 4R)
    whht = np.ascontiguousarray(W_hh.T, dtype=f16)         # (R, 3R)
    wlht = np.ascontiguousarray(W_lh.T, dtype=f16)         # (ZC, R)
    woutt = np.ascontiguousarray(W_out.T, dtype=f16)       # (R, NCH)
    bzl = np.ascontiguousarray(b_z.reshape(KT, 128), dtype=f16)
    blhl = np.ascontiguousarray(b_lh.reshape(KT, 128), dtype=f16)
    boutr = np.ascontiguousarray(b_out.reshape(NCH, 1).astype(np.float32))
    sel2 = np.zeros((KT, KT * BC), dtype=f16)
    for c in range(KT):
        sel2[c, c * BC : (c + 1) * BC] = 1.0
    zct_full = np.concatenate([z, x_cond], axis=1).T.astype(f16)  # (ZC, B)

    if "nc" not in _CACHE:
        _CACHE["nc"] = _build()
    nc = _CACHE["nc"]

    in_maps = []
    for c in range(NCORES):
        in_maps.append({
            "zct": np.ascontiguousarray(zct_full[:, c * BC : (c + 1) * BC]),
            "wft": wft,
            "whht": whht,
            "wlht": wlht,
            "woutt": woutt,
            "biases": biases,
            "bzl": bzl,
            "blhl": blhl,
            "bout": boutr,
            "sel2": sel2,
        })

    global _LAST_IN_MAPS
    _LAST_IN_MAPS = in_maps
    res = run_bass_kernel_spmd(nc, in_maps, core_ids=list(range(NCORES)))
    # per-core out: (MAXLEN, NCH, BC) -> (BC, MAXLEN, NCH)
    parts = [np.asarray(res.results[c]["out"]).transpose(2, 0, 1)
             for c in range(NCORES)]
    return np.ascontiguousarray(np.concatenate(parts, axis=0), dtype=np.float32)


# revision 23
# speedup vs baseline: 1.3519x; 1.3519x over previous
"""Trainium2 Bass kernel for a conditional GRU decoder.

Model (per reference):
  h0 = [z, x_cond] @ W_lh.T + b_lh
  x0 = 0
  for t in 0..127:
      hn = GRUCell(x_t, h_t);  logits_t = hn @ W_out.T + b_out;  x_{t+1} = hn
  out = (B, 128, 64)

Because x_{t+1} == h_{t+1} for t >= 1, the two GRU matmuls fuse into one
(B,R) @ (R,4R) matmul with W_fused rows [Wi_r+Wh_r; Wi_z+Wh_z; Wi_n; Wh_n].

Sharding: data-parallel over batch, B=2048 -> 8 cores x 256. All weights
replicated. On-chip layout is transposed (feature dim on partitions, batch on
the free dim) so gate biases are per-partition scalars and the recurrent
matmuls keep weights stationary:
    gatesT[4R, b] = W_fused @ hT   via  matmul(out, lhsT=W_fused.T, rhs=hT)

Engine split per step (chunk c = feature rows c*128..c*128+127):
  PE:   16 gate MMs (N=256 f16) + logits (2 MMs + K=1 ones-MM injecting
        b_out into PSUM, so logits DMA straight from PSUM with no ACT pass)
        + one K=2 selector bias-MM injecting b_z into the gz bank (so the
        u-sigmoid is a single [128,512] ACT instr).
  ACT:  r = sigmoid per chunk (bias in ACT), u = sigmoid [128,512],
        n = tanh per chunk (bias pre-added by the pre-STT).
  DVE:  t2_c=(b_hn+ghn)*r (STT), pre_c=(b_in+gin)+t2 (STT),
        p1n_c=(u-1)*n (STT f16), h'_0=p2_0-p1n_0 (chain chunk).
  Pool: p2_c=u*h, h'_1 (off-chain chunk, staggered for the k1 MMs).

All matmul operands f16 (measured end-to-end rel err ~1e-3 scale); gate
accumulation and logits stay f32 in PSUM.
"""

import numpy as np

import concourse.bass as bass
import concourse.tile as tile
from concourse import bacc, mybir
from concourse.bass_utils import run_bass_kernel_spmd

F32 = mybir.dt.float32
F16 = mybir.dt.float16
ACT = mybir.ActivationFunctionType
ALU = mybir.AluOpType

B = 2048
HID = 256
COND = 128
NCH = 64
MAXLEN = 128
R = 256
NCORES = 8
BC = B // NCORES  # 256 per-core batch
KT = R // 128     # 2 k-tiles over R
ZC = HID + COND   # 384
ZKT = ZC // 128   # 3 k-tiles over hid+cond


def _build(maxlen=MAXLEN, dbg=False):
    nc = bacc.Bacc("TRN2", target_bir_lowering=False, debug=False)

    # ---- DRAM I/O (per-core shapes) ----
    d_zct = nc.dram_tensor("zct", [ZC, BC], F16, kind="ExternalInput")
    d_wf = nc.dram_tensor("wft", [R, 4 * R], F16, kind="ExternalInput")
    d_whh = nc.dram_tensor("whht", [R, 3 * R], F16, kind="ExternalInput")
    d_wlh = nc.dram_tensor("wlht", [ZC, R], F16, kind="ExternalInput")
    d_wout = nc.dram_tensor("woutt", [R, NCH], F16, kind="ExternalInput")
    # per-partition bias columns: 0,1=b_r  2,3=b_hn  4,5=b_in (per chunk)
    d_bias = nc.dram_tensor("biases", [128, 6], F32, kind="ExternalInput")
    # selector-MM operands
    d_bzl = nc.dram_tensor("bzl", [KT, 128], F16, kind="ExternalInput")
    d_blhl = nc.dram_tensor("blhl", [KT, 128], F16, kind="ExternalInput")
    d_bout = nc.dram_tensor("bout", [NCH, 1], F32, kind="ExternalInput")
    d_sel2 = nc.dram_tensor("sel2", [KT, KT * BC], F16, kind="ExternalInput")
    d_out = nc.dram_tensor("out", [maxlen, NCH, BC], F32, kind="ExternalOutput")
    d_ka = nc.dram_tensor("kadbg", [128, BC], F32, kind="ExternalOutput")
    d_dbg = {}
    if dbg:
        for nm in ("h0", "r", "u", "t2", "nt", "p2", "p1n", "hs", "gr",
                   "ghn"):
            d_dbg[nm] = nc.dram_tensor("dbg_" + nm, [128, KT, BC], F32,
                                       kind="ExternalOutput")

    with tile.TileContext(nc) as tc:
        with (
            tc.tile_pool(name="const", bufs=1) as const,
            tc.tile_pool(name="state", bufs=1) as state,
            tc.tile_pool(name="ew", bufs=2) as ew,
            tc.tile_pool(name="pgr", bufs=1, space="PSUM") as pgr,
            tc.tile_pool(name="pghn", bufs=1, space="PSUM") as pghn,
            tc.tile_pool(name="pgin", bufs=1, space="PSUM") as pgin,
            tc.tile_pool(name="pgz", bufs=1, space="PSUM") as pgz,
            tc.tile_pool(name="pl", bufs=2, space="PSUM") as pl,
            tc.tile_pool(name="ph0", bufs=1, space="PSUM") as ph0,
        ):
            # ---- load constants ----
            wf = const.tile([128, KT, 4 * R], F16)
            nc.sync.dma_start(wf, d_wf[:].rearrange("(k p) m -> p k m", p=128))
            whh = const.tile([128, KT, 3 * R], F16)
            nc.sync.dma_start(whh, d_whh[:].rearrange("(k p) m -> p k m", p=128))
            wlh = const.tile([128, ZKT, R], F16)
            nc.sync.dma_start(wlh, d_wlh[:].rearrange("(k p) m -> p k m", p=128))
            wout = const.tile([128, KT, NCH], F16)
            nc.sync.dma_start(wout, d_wout[:].rearrange("(k p) m -> p k m", p=128))
            zct = const.tile([128, ZKT, BC], F16)
            nc.sync.dma_start(zct, d_zct[:].rearrange("(k p) m -> p k m", p=128))
            bia = const.tile([128, 6], F32)
            nc.sync.dma_start(bia, d_bias[:])
            bzl = const.tile([KT, 128], F16)
            nc.sync.dma_start(bzl, d_bzl[:])
            blhl = const.tile([KT, 128], F16)
            nc.sync.dma_start(blhl, d_blhl[:])
            boutc = const.tile([NCH, 1], F32)
            nc.sync.dma_start(boutc, d_bout[:])
            sel2 = const.tile([KT, KT * BC], F16)
            nc.sync.dma_start(sel2, d_sel2[:])

            def bcol(i):
                return bia[:, i : i + 1]

            h = state.tile([128, KT, BC], F16)  # hT, chunk c = rows c*128..

            # PE keepalive: dummy accumulating matmuls on chain tiles keep
            # the HAM clock-gate at full rate through the per-step EW gap.
            ka = ph0.tile([128, KT, BC], F32, tag="ka")
            ka_started = [False]

            def keepalive_on(rhs):
                n = rhs.free_size()
                nc.tensor.matmul(ka[:, 0, 0:n] if n <= BC else ka[:],
                                 wf[:, 0, 0:128], rhs,
                                 start=(not ka_started[0]), stop=False,
                                 skip_group_check=True)
                ka_started[0] = True

            # ---- HAM warmup: ~4.5us of back-to-back MMs so the SHORT
            # window fires and the PE clock un-throttles to 2.4 GHz; the
            # per-step keepalives then keep every MID window non-idle.
            for _ in range(11):
                keepalive_on(zct[:, 0:2, :])

            # ---- h0 = W_lh @ zcT + b_lh (selector bias-MM + 3 k-tiles) ----
            ph = ph0.tile([128, KT, BC], F32, tag="h0")
            nc.tensor.matmul(ph[:], blhl[:], sel2[:],
                             start=True, stop=False, skip_group_check=True)
            for m in range(KT):
                for k in range(ZKT):
                    nc.tensor.matmul(ph[:, m, :], wlh[:, k, bass.ts(m, 128)],
                                     zct[:, k, :], start=False,
                                     stop=(m == KT - 1 and k == ZKT - 1),
                                     skip_group_check=True)
            nc.scalar.activation(h[:],
                                 ph[:], ACT.Identity,
                                 bias=0.0)
            dbg_f32 = {}
            if dbg:
                for nm in d_dbg:
                    dbg_f32[nm] = const.tile([128, KT, BC], F32,
                                             name="dbgt_" + nm)
                nc.vector.tensor_copy(dbg_f32["h0"], h[:])
                nc.sync.dma_start(d_dbg["h0"][:], dbg_f32["h0"])

            # pending logits emitters from the previous step
            pending = []

            def flush_logits():
                for fn in pending:
                    fn()
                pending.clear()

            def emit_step(t, first):
                if first:
                    # x=0: gates come from W_hh only (pytorch order r,z,n)
                    w, offs = whh, {"r": 0, "z": R, "hn": 2 * R}
                else:
                    w, offs = wf, {"r": 0, "z": R, "in": 2 * R, "hn": 3 * R}

                gr = pgr.tile([128, KT, BC], F32, tag="gr")
                ghn = pghn.tile([128, KT, BC], F32, tag="ghn")
                gin = pgin.tile([128, KT, BC], F32, tag="gin")
                gz = pgz.tile([128, KT, BC], F32, tag="gz")

                def gate_mm(dst, name, m, k, start, stop):
                    nc.tensor.matmul(
                        dst[:, m, :], w[:, k, bass.ds(offs[name] + m * 128, 128)],
                        h[:, k, :], start=start, stop=stop,
                        skip_group_check=True)

                # ---- PE burst (gate-major: r completes first so the
                # chain-head sigmoids unblock ~700ns earlier) ----
                # gz bias-MM first: no h dependency, runs during prev EW tail
                nc.tensor.matmul(gz[:], bzl[:], sel2[:],
                                 start=True, stop=False, skip_group_check=True)
                names = ("r", "hn") if first else ("r", "hn", "in")
                for name in names + ("z",):
                    dst = {"r": gr, "hn": ghn, "in": gin, "z": gz}[name]
                    for m in range(KT):
                        for k in range(KT):
                            gate_mm(dst, name, m, k,
                                    start=(name != "z" and m == 0 and k == 0),
                                    stop=(m == KT - 1 and k == KT - 1))
                # pending logits (consume h(t-1), off the chain-critical path)
                flush_logits()
                # t2 -> gin bank via identity-weight MMs (replaces pre-STT)
                # (emitted after the elementwise t2 below via a closure list)

                if dbg and t == 0:
                    nc.vector.tensor_copy(dbg_f32["gr"], gr[:])
                    nc.sync.dma_start(d_dbg["gr"][:], dbg_f32["gr"])
                    nc.vector.tensor_copy(dbg_f32["ghn"], ghn[:])
                    nc.sync.dma_start(d_dbg["ghn"][:], dbg_f32["ghn"])
                # ---- ACT: r per chunk (chain head), u merged ----
                r = ew.tile([128, KT, BC], F16, tag="r")
                with tc.high_priority(64):
                    for m in range(KT):
                        nc.scalar.activation(r[:, m, :], gr[:, m, :],
                                             ACT.Sigmoid, bias=bcol(0 + m))
                u = ew.tile([128, KT, BC], F16, tag="u")
                nc.scalar.activation(u[:],
                                     gz[:], ACT.Sigmoid,
                                     bias=0.0)

                # ---- DVE: t2 = (b_hn + ghn) * r, pre = (b_in + gin) + t2
                t2 = ew.tile([128, KT, BC], F16, tag="t2")
                pre = ew.tile([128, KT, BC], F16, tag="pre")
                nt = ew.tile([128, KT, BC], F16, tag="nt")
                for m in range(KT):
                    nc.vector.scalar_tensor_tensor(
                        t2[:, m, :], ghn[:, m, :], bcol(2 + m), r[:, m, :],
                        op0=ALU.add, op1=ALU.mult)
                    if first:
                        nc.vector.tensor_scalar_add(pre[:, m, :], t2[:, m, :],
                                                    bcol(4 + m))
                    else:
                        nc.vector.scalar_tensor_tensor(
                            pre[:, m, :], gin[:, m, :], bcol(4 + m),
                            t2[:, m, :], op0=ALU.add, op1=ALU.add)
                    nc.scalar.activation(nt[:, m, :], pre[:, m, :], ACT.Tanh,
                                         bias=0.0)

                # ---- DVE chain: (after tanh) p1n, h' ----
                p1n = ew.tile([128, KT, BC], F16, tag="p1n")
                p2 = ew.tile([128, KT, BC], F16, tag="p2")
                # p2 = u*h on Pool (off-chain; reads old h)
                for m in range(KT):
                    nc.gpsimd.tensor_mul(p2[:, m, :], u[:, m, :], h[:, m, :])
                # p1n = (u-1)*n ; h' = p2 - p1n = u*h + (1-u)*n
                for m in range(KT):
                    nc.vector.scalar_tensor_tensor(
                        p1n[:, m, :], u[:, m, :], -1.0, nt[:, m, :],
                        op0=ALU.add, op1=ALU.mult)
                    nc.vector.tensor_sub(h[:, m, :], p2[:, m, :],
                                         p1n[:, m, :])
                if dbg and t == 0:
                    for nm, tl in (("r", r), ("u", u), ("t2", t2),
                                   ("nt", nt), ("p2", p2),
                                   ("p1n", p1n), ("hs", h)):
                        nc.vector.tensor_copy(dbg_f32[nm], tl[:])
                        nc.sync.dma_start(d_dbg[nm][:], dbg_f32[nm])

                # keepalives spread through the EW tail (fat N=512 MMs
                # keep the PE duty cycle high enough that HAM stays warm)
                keepalive_on(r[:])
                keepalive_on(t2[:, 0, :])
                keepalive_on(t2[:])
                keepalive_on(pre[:, 0, :])
                keepalive_on(pre[:])
                keepalive_on(nt[:, 0, :])
                keepalive_on(nt[:])
                keepalive_on(p1n[:])

                # ---- logits (deferred into next burst) ----
                lp = pl.tile([NCH, BC], F32, tag="lp")

                def do_logits(lp=lp, t=t):
                    for k in range(KT):
                        nc.tensor.matmul(lp, wout[:, k, :], h[:, k, :],
                                         start=(k == 0), stop=(k == KT - 1),
                                         skip_group_check=True)
                    ls = ew.tile([NCH, BC], F32, tag="ls")
                    nc.scalar.activation(ls, lp, ACT.Identity,
                                         bias=boutc[:, 0:1])
                    nc.sync.dma_start(d_out[t], ls)
                pending.append(do_logits)

            emit_step(0, first=True)
            for t in range(1, maxlen):
                emit_step(t, first=False)
            flush_logits()
            kcopy = ew.tile([128, BC], F32, tag="kcopy")
            nc.scalar.activation(kcopy, ka[:, 0, :], ACT.Identity, bias=0.0)
            nc.sync.dma_start(d_ka[:], kcopy)

    nc.compile()
    return nc


_CACHE = {}
_LAST_IN_MAPS = None


def kernel(z, x_cond, W_lh, b_lh, W_ih, W_hh, b_ih, b_hh, W_out, b_out):
    z = np.asarray(z, np.float32)
    x_cond = np.asarray(x_cond, np.float32)
    W_lh = np.asarray(W_lh, np.float32)
    b_lh = np.asarray(b_lh, np.float32)
    W_ih = np.asarray(W_ih, np.float32)
    W_hh = np.asarray(W_hh, np.float32)
    b_ih = np.asarray(b_ih, np.float32)
    b_hh = np.asarray(b_hh, np.float32)
    W_out = np.asarray(W_out, np.float32)
    b_out = np.asarray(b_out, np.float32)

    # fused recurrent weight: rows [Wi_r+Wh_r; Wi_z+Wh_z; Wi_n; Wh_n]
    Wf = np.concatenate(
        [W_ih[:R] + W_hh[:R], W_ih[R : 2 * R] + W_hh[R : 2 * R],
         W_ih[2 * R :], W_hh[2 * R :]], axis=0)
    b_r = b_ih[:R] + b_hh[:R]
    b_z = b_ih[R : 2 * R] + b_hh[R : 2 * R]
    b_in = b_ih[2 * R :]
    b_hn = b_hh[2 * R :]

    def pcols(v):  # (R,) -> (128, KT) per-partition columns
        return np.ascontiguousarray(v.reshape(KT, 128).T)

    biases = np.ascontiguousarray(
        np.concatenate([pcols(b_r), pcols(b_hn), pcols(b_in)],
                       axis=1))  # (128, 6)

    f16 = np.float16
    wft = np.ascontiguousarray(Wf.T, dtype=f16)            # (R, 4R)
    whht = np.ascontiguousarray(W_hh.T, dtype=f16)         # (R, 3R)
    wlht = np.ascontiguousarray(W_lh.T, dtype=f16)         # (ZC, R)
    woutt = np.ascontiguousarray(W_out.T, dtype=f16)       # (R, NCH)
    bzl = np.ascontiguousarray(b_z.reshape(KT, 128), dtype=f16)
    blhl = np.ascontiguousarray(b_lh.reshape(KT, 128), dtype=f16)
    boutr = np.ascontiguousarray(b_out.reshape(NCH, 1).astype(np.float32))
    sel2 = np.zeros((KT, KT * BC), dtype=f16)
    for c in range(KT):
        sel2[c, c * BC : (c + 1) * BC] = 1.0
    zct_full = np.concatenate([z, x_cond], axis=1).T.astype(f16)  # (ZC, B)

    if "nc" not in _CACHE:
        _CACHE["nc"] = _build()
    nc = _CACHE["nc"]

    in_maps = []
    for c in range(NCORES):
        in_maps.append({
            "zct": np.ascontiguousarray(zct_full[:, c * BC : (c + 1) * BC]),
            "wft": wft,
            "whht": whht,
            "wlht": wlht,
            "woutt": woutt,
            "biases": biases,
            "bzl": bzl,
            "blhl": blhl,
            "bout": boutr,
            "sel2": sel2,
        })

    global _LAST_IN_MAPS
    _LAST_IN_MAPS = in_maps
    res = run_bass_kernel_spmd(nc, in_maps, core_ids=list(range(NCORES)))
    # per-core out: (MAXLEN, NCH, BC) -> (BC, MAXLEN, NCH)
    parts = [np.asarray(res.results[c]["out"]).transpose(2, 0, 1)
             for c in range(NCORES)]
    return np.ascontiguousarray(np.concatenate(parts, axis=0), dtype=np.float32)


# revision 25
# speedup vs baseline: 1.3960x; 1.0326x over previous
"""Trainium2 Bass kernel for a conditional GRU decoder.

Model (per reference):
  h0 = [z, x_cond] @ W_lh.T + b_lh
  x0 = 0
  for t in 0..127:
      hn = GRUCell(x_t, h_t);  logits_t = hn @ W_out.T + b_out;  x_{t+1} = hn
  out = (B, 128, 64)

Because x_{t+1} == h_{t+1} for t >= 1, the two GRU matmuls fuse into one
(B,R) @ (R,4R) matmul with W_fused rows [Wi_r+Wh_r; Wi_z+Wh_z; Wi_n; Wh_n].

Sharding: data-parallel over batch, B=2048 -> 8 cores x 256. All weights
replicated. On-chip layout is transposed (feature dim on partitions, batch on
the free dim) so gate biases are per-partition scalars and the recurrent
matmuls keep weights stationary:
    gatesT[4R, b] = W_fused @ hT   via  matmul(out, lhsT=W_fused.T, rhs=hT)

Engine split per step (chunk c = feature rows c*128..c*128+127):
  PE:   16 gate MMs (N=256 f16) + logits (2 MMs + K=1 ones-MM injecting
        b_out into PSUM, so logits DMA straight from PSUM with no ACT pass)
        + one K=2 selector bias-MM injecting b_z into the gz bank (so the
        u-sigmoid is a single [128,512] ACT instr).
  ACT:  r = sigmoid per chunk (bias in ACT), u = sigmoid [128,512],
        n = tanh per chunk (bias pre-added by the pre-STT).
  DVE:  t2_c=(b_hn+ghn)*r (STT), pre_c=(b_in+gin)+t2 (STT),
        p1n_c=(u-1)*n (STT f16), h'_0=p2_0-p1n_0 (chain chunk).
  Pool: p2_c=u*h, h'_1 (off-chain chunk, staggered for the k1 MMs).

All matmul operands f16 (measured end-to-end rel err ~1e-3 scale); gate
accumulation and logits stay f32 in PSUM.
"""

import numpy as np

import concourse.bass as bass
import concourse.tile as tile
from concourse import bacc, mybir
from concourse.bass_utils import run_bass_kernel_spmd

F32 = mybir.dt.float32
F16 = mybir.dt.float16
ACT = mybir.ActivationFunctionType
ALU = mybir.AluOpType

B = 2048
HID = 256
COND = 128
NCH = 64
MAXLEN = 128
R = 256
NCORES = 8
BC = B // NCORES  # 256 per-core batch
KT = R // 128     # 2 k-tiles over R
ZC = HID + COND   # 384
ZKT = ZC // 128   # 3 k-tiles over hid+cond


def _build(maxlen=MAXLEN, dbg=False):
    nc = bacc.Bacc("TRN2", target_bir_lowering=False, debug=False)

    # ---- DRAM I/O (per-core shapes) ----
    d_zct = nc.dram_tensor("zct", [ZC, BC], F16, kind="ExternalInput")
    d_wf = nc.dram_tensor("wft", [R, 4 * R], F16, kind="ExternalInput")
    d_whh = nc.dram_tensor("whht", [R, 3 * R], F16, kind="ExternalInput")
    d_wlh = nc.dram_tensor("wlht", [ZC, R], F16, kind="ExternalInput")
    d_wout = nc.dram_tensor("woutt", [R, NCH], F16, kind="ExternalInput")
    # per-partition bias columns: 0,1=b_r  2,3=b_hn  4,5=b_in (per chunk)
    d_bias = nc.dram_tensor("biases", [128, 6], F32, kind="ExternalInput")
    # selector-MM operands
    d_bzl = nc.dram_tensor("bzl", [KT, 128], F16, kind="ExternalInput")
    d_blhl = nc.dram_tensor("blhl", [KT, 128], F16, kind="ExternalInput")
    d_bout = nc.dram_tensor("bout", [NCH, 1], F32, kind="ExternalInput")
    d_sel2 = nc.dram_tensor("sel2", [KT, KT * BC], F16, kind="ExternalInput")
    d_out = nc.dram_tensor("out", [maxlen, NCH, BC], F32, kind="ExternalOutput")
    d_ka = nc.dram_tensor("kadbg", [128, BC], F32, kind="ExternalOutput")
    d_dbg = {}
    if dbg:
        for nm in ("h0", "r", "u", "t2", "nt", "p2", "p1n", "hs", "gr",
                   "ghn"):
            d_dbg[nm] = nc.dram_tensor("dbg_" + nm, [128, KT, BC], F32,
                                       kind="ExternalOutput")

    with tile.TileContext(nc) as tc:
        with (
            tc.tile_pool(name="const", bufs=1) as const,
            tc.tile_pool(name="state", bufs=1) as state,
            tc.tile_pool(name="ew", bufs=2) as ew,
            tc.tile_pool(name="pgr", bufs=1, space="PSUM") as pgr,
            tc.tile_pool(name="pghn", bufs=1, space="PSUM") as pghn,
            tc.tile_pool(name="pgin", bufs=1, space="PSUM") as pgin,
            tc.tile_pool(name="pgz", bufs=1, space="PSUM") as pgz,
            tc.tile_pool(name="pl", bufs=2, space="PSUM") as pl,
            tc.tile_pool(name="ph0", bufs=1, space="PSUM") as ph0,
        ):
            # ---- load constants ----
            wf = const.tile([128, KT, 4 * R], F16)
            nc.sync.dma_start(wf, d_wf[:].rearrange("(k p) m -> p k m", p=128))
            whh = const.tile([128, KT, 3 * R], F16)
            nc.sync.dma_start(whh, d_whh[:].rearrange("(k p) m -> p k m", p=128))
            wlh = const.tile([128, ZKT, R], F16)
            nc.sync.dma_start(wlh, d_wlh[:].rearrange("(k p) m -> p k m", p=128))
            wout = const.tile([128, KT, NCH], F16)
            nc.sync.dma_start(wout, d_wout[:].rearrange("(k p) m -> p k m", p=128))
            zct = const.tile([128, ZKT, BC], F16)
            nc.sync.dma_start(zct, d_zct[:].rearrange("(k p) m -> p k m", p=128))
            bia = const.tile([128, 6], F32)
            nc.sync.dma_start(bia, d_bias[:])
            bzl = const.tile([KT, 128], F16)
            nc.sync.dma_start(bzl, d_bzl[:])
            blhl = const.tile([KT, 128], F16)
            nc.sync.dma_start(blhl, d_blhl[:])
            boutc = const.tile([NCH, 1], F32)
            nc.sync.dma_start(boutc, d_bout[:])
            sel2 = const.tile([KT, KT * BC], F16)
            nc.sync.dma_start(sel2, d_sel2[:])

            def bcol(i):
                return bia[:, i : i + 1]

            h = state.tile([128, KT, BC], F16)  # hT, chunk c = rows c*128..

            # PE keepalive: dummy accumulating matmuls on chain tiles keep
            # the HAM clock-gate at full rate through the per-step EW gap.
            ka = ph0.tile([128, KT, BC], F32, tag="ka")
            ka_started = [False]

            def keepalive_on(rhs):
                n = rhs.free_size()
                nc.tensor.matmul(ka[:, 0, 0:n] if n <= BC else ka[:],
                                 wf[:, 0, 0:128], rhs,
                                 start=(not ka_started[0]), stop=False,
                                 skip_group_check=True)
                ka_started[0] = True

            # ---- HAM warmup: ~4.5us of back-to-back MMs so the SHORT
            # window fires and the PE clock un-throttles to 2.4 GHz; the
            # per-step keepalives then keep every MID window non-idle.
            for _ in range(11):
                keepalive_on(zct[:, 0:2, :])

            # ---- h0 = W_lh @ zcT + b_lh (selector bias-MM + 3 k-tiles) ----
            ph = ph0.tile([128, KT, BC], F32, tag="h0")
            nc.tensor.matmul(ph[:], blhl[:], sel2[:],
                             start=True, stop=False, skip_group_check=True)
            for m in range(KT):
                for k in range(ZKT):
                    nc.tensor.matmul(ph[:, m, :], wlh[:, k, bass.ts(m, 128)],
                                     zct[:, k, :], start=False,
                                     stop=(m == KT - 1 and k == ZKT - 1),
                                     skip_group_check=True)
            nc.scalar.activation(h[:],
                                 ph[:], ACT.Identity,
                                 bias=0.0)
            dbg_f32 = {}
            if dbg:
                for nm in d_dbg:
                    dbg_f32[nm] = const.tile([128, KT, BC], F32,
                                             name="dbgt_" + nm)
                nc.vector.tensor_copy(dbg_f32["h0"], h[:])
                nc.sync.dma_start(d_dbg["h0"][:], dbg_f32["h0"])

            # pending logits emitters from the previous step
            pending = []

            def flush_logits():
                for fn in pending:
                    fn()
                pending.clear()

            def emit_step(t, first):
                if first:
                    # x=0: gates come from W_hh only (pytorch order r,z,n)
                    w, offs = whh, {"r": 0, "z": R, "hn": 2 * R}
                else:
                    w, offs = wf, {"r": 0, "z": R, "in": 2 * R, "hn": 3 * R}

                gr = pgr.tile([128, KT, BC], F32, tag="gr")
                ghn = pghn.tile([128, KT, BC], F32, tag="ghn")
                gin = pgin.tile([128, KT, BC], F32, tag="gin")
                gz = pgz.tile([128, KT, BC], F32, tag="gz")

                def gate_mm(dst, name, m, k, start, stop):
                    nc.tensor.matmul(
                        dst[:, m, :], w[:, k, bass.ds(offs[name] + m * 128, 128)],
                        h[:, k, :], start=start, stop=stop,
                        skip_group_check=True)

                # ---- PE burst (gate-major: r completes first so the
                # chain-head sigmoids unblock ~700ns earlier) ----
                # gz bias-MM first: no h dependency, runs during prev EW tail
                nc.tensor.matmul(gz[:], bzl[:], sel2[:],
                                 start=True, stop=False, skip_group_check=True)
                names = ("r", "hn") if first else ("r", "hn", "in")
                for name in names + ("z",):
                    dst = {"r": gr, "hn": ghn, "in": gin, "z": gz}[name]
                    for m in range(KT):
                        for k in range(KT):
                            gate_mm(dst, name, m, k,
                                    start=(name != "z" and m == 0 and k == 0),
                                    stop=(m == KT - 1 and k == KT - 1))
                # t2 -> gin bank via identity-weight MMs (replaces pre-STT)
                # (emitted after the elementwise t2 below via a closure list)

                if dbg and t == 0:
                    nc.vector.tensor_copy(dbg_f32["gr"], gr[:])
                    nc.sync.dma_start(d_dbg["gr"][:], dbg_f32["gr"])
                    nc.vector.tensor_copy(dbg_f32["ghn"], ghn[:])
                    nc.sync.dma_start(d_dbg["ghn"][:], dbg_f32["ghn"])
                # ---- ACT: r per chunk (chain head), u merged ----
                r = ew.tile([128, KT, BC], F16, tag="r")
                with tc.high_priority(64):
                    for m in range(KT):
                        nc.scalar.activation(r[:, m, :], gr[:, m, :],
                                             ACT.Sigmoid, bias=bcol(0 + m))
                u = ew.tile([128, KT, BC], F16, tag="u")
                nc.scalar.activation(u[:],
                                     gz[:], ACT.Sigmoid,
                                     bias=0.0)

                # ---- DVE: t2 = (b_hn + ghn) * r, pre = (b_in + gin) + t2
                t2 = ew.tile([128, KT, BC], F16, tag="t2")
                pre = ew.tile([128, KT, BC], F16, tag="pre")
                nt = ew.tile([128, KT, BC], F16, tag="nt")
                for m in range(KT):
                    nc.vector.scalar_tensor_tensor(
                        t2[:, m, :], ghn[:, m, :], bcol(2 + m), r[:, m, :],
                        op0=ALU.add, op1=ALU.mult)
                    if first:
                        nc.vector.tensor_scalar_add(pre[:, m, :], t2[:, m, :],
                                                    bcol(4 + m))
                    else:
                        nc.vector.scalar_tensor_tensor(
                            pre[:, m, :], gin[:, m, :], bcol(4 + m),
                            t2[:, m, :], op0=ALU.add, op1=ALU.add)
                    nc.scalar.activation(nt[:, m, :], pre[:, m, :], ACT.Tanh,
                                         bias=0.0)

                # pending logits from the previous step: must be emitted
                # before this step's h'-writes (program order fixes which h
                # version the MMs read), but sits late in the PE/ACT queues
                flush_logits()

                # ---- combine: p2 = u*h, p1n = (u-1)*n, h' = p2 - p1n ----
                p1n = ew.tile([128, KT, BC], F16, tag="p1n")
                p2 = ew.tile([128, KT, BC], F16, tag="p2")
                # p2 chunk0 on DVE (fits in the tanh-0 window), chunk1 Pool
                nc.vector.tensor_mul(p2[:, 0, :], u[:, 0, :], h[:, 0, :])
                nc.gpsimd.tensor_mul(p2[:, 1, :], u[:, 1, :], h[:, 1, :])
                with tc.high_priority(64):
                    nc.vector.scalar_tensor_tensor(
                        p1n[:, 0, :], u[:, 0, :], -1.0, nt[:, 0, :],
                        op0=ALU.add, op1=ALU.mult)
                    nc.vector.tensor_sub(h[:, 0, :], p2[:, 0, :],
                                         p1n[:, 0, :])
                nc.vector.scalar_tensor_tensor(
                    p1n[:, 1, :], u[:, 1, :], -1.0, nt[:, 1, :],
                    op0=ALU.add, op1=ALU.mult)
                nc.vector.tensor_sub(h[:, 1, :], p2[:, 1, :], p1n[:, 1, :])
                if dbg and t == 0:
                    for nm, tl in (("r", r), ("u", u), ("t2", t2),
                                   ("nt", nt), ("p2", p2),
                                   ("p1n", p1n), ("hs", h)):
                        nc.vector.tensor_copy(dbg_f32[nm], tl[:])
                        nc.sync.dma_start(d_dbg[nm][:], dbg_f32[nm])

                # keepalives spread through the EW tail (fat N=512 MMs
                # keep the PE duty cycle high enough that HAM stays warm)
                keepalive_on(r[:])
                keepalive_on(t2[:, 0, :])
                keepalive_on(t2[:])
                keepalive_on(pre[:, 0, :])
                keepalive_on(pre[:])
                keepalive_on(nt[:, 0, :])
                keepalive_on(nt[:])
                keepalive_on(p1n[:])

                # ---- logits (deferred into next burst) ----
                lp = pl.tile([NCH, BC], F32, tag="lp")

                def do_logits(lp=lp, t=t):
                    for k in range(KT):
                        nc.tensor.matmul(lp, wout[:, k, :], h[:, k, :],
                                         start=(k == 0), stop=(k == KT - 1),
                                         skip_group_check=True)
                    ls = ew.tile([NCH, BC], F32, tag="ls")
                    nc.scalar.activation(ls, lp, ACT.Identity,
                                         bias=boutc[:, 0:1])
                    nc.sync.dma_start(d_out[t], ls)
                pending.append(do_logits)

            emit_step(0, first=True)
            for t in range(1, maxlen):
                emit_step(t, first=False)
            flush_logits()
            kcopy = ew.tile([128, BC], F32, tag="kcopy")
            nc.scalar.activation(kcopy, ka[:, 0, :], ACT.Identity, bias=0.0)
            nc.sync.dma_start(d_ka[:], kcopy)

    nc.compile()
    return nc


_CACHE = {}
_LAST_IN_MAPS = None


def kernel(z, x_cond, W_lh, b_lh, W_ih, W_hh, b_ih, b_hh, W_out, b_out):
    z = np.asarray(z, np.float32)
    x_cond = np.asarray(x_cond, np.float32)
    W_lh = np.asarray(W_lh, np.float32)
    b_lh = np.asarray(b_lh, np.float32)
    W_ih = np.asarray(W_ih, np.float32)
    W_hh = np.asarray(W_hh, np.float32)
    b_ih = np.asarray(b_ih, np.float32)
    b_hh = np.asarray(b_hh, np.float32)
    W_out = np.asarray(W_out, np.float32)
    b_out = np.asarray(b_out, np.float32)

    # fused recurrent weight: rows [Wi_r+Wh_r; Wi_z+Wh_z; Wi_n; Wh_n]
    Wf = np.concatenate(
        [W_ih[:R] + W_hh[:R], W_ih[R : 2 * R] + W_hh[R : 2 * R],
         W_ih[2 * R :], W_hh[2 * R :]], axis=0)
    b_r = b_ih[:R] + b_hh[:R]
    b_z = b_ih[R : 2 * R] + b_hh[R : 2 * R]
    b_in = b_ih[2 * R :]
    b_hn = b_hh[2 * R :]

    def pcols(v):  # (R,) -> (128, KT) per-partition columns
        return np.ascontiguousarray(v.reshape(KT, 128).T)

    biases = np.ascontiguousarray(
        np.concatenate([pcols(b_r), pcols(b_hn), pcols(b_in)],
                       axis=1))  # (128, 6)

    f16 = np.float16
    wft = np.ascontiguousarray(Wf.T, dtype=f16)            # (R, 4R)
    whht = np.ascontiguousarray(W_hh.T, dtype=f16)         # (R, 3R)
    wlht = np.ascontiguousarray(W_lh.T, dtype=f16)         # (ZC, R)
    woutt = np.ascontiguousarray(W_out.T, dtype=f16)       # (R, NCH)
    bzl = np.ascontiguousarray(b_z.reshape(KT, 128), dtype=f16)
    blhl = np.ascontiguousarray(b_lh.reshape(KT, 128), dtype=f16)
    boutr = np.ascontiguousarray(b_out.reshape(NCH, 1).astype(np.float32))
    sel2 = np.zeros((KT, KT * BC), dtype=f16)
    for c in range(KT):
        sel2[c, c * BC : (c + 1) * BC] = 1.0
    zct_full = np.concatenate([z, x_cond], axis=1).T.astype(f16)  # (ZC, B)

    if "nc" not in _CACHE:
        _CACHE["nc"] = _build()
    nc = _CACHE["nc"]

    in_maps = []
    for c in range(NCORES):
        in_maps.append({
            "zct": np.ascontiguousarray(zct_full[:, c * BC : (c + 1) * BC]),
            "wft": wft,
            "whht": whht,
            "wlht": wlht,
            "woutt": woutt,
            "biases": biases,
            "bzl": bzl,
            "blhl": blhl,
            "bout": boutr,
            "sel2": sel2,
        })

    global _LAST_IN_MAPS
    _LAST_IN_MAPS = in_maps
    res = run_bass_kernel_spmd(nc, in_maps, core_ids=list(range(NCORES)))
    # per-core out: (MAXLEN, NCH, BC) -> (BC, MAXLEN, NCH)
    parts = [np.asarray(res.results[c]["out"]).transpose(2, 0, 1)
             for c in range(NCORES)]
    return np.ascontiguousarray(np.concatenate(parts, axis=0), dtype=np.float32)


# revision 33
# speedup vs baseline: 1.4000x; 1.0029x over previous
"""Trainium2 Bass kernel for a conditional GRU decoder.

Model (per reference):
  h0 = [z, x_cond] @ W_lh.T + b_lh
  x0 = 0
  for t in 0..127:
      hn = GRUCell(x_t, h_t);  logits_t = hn @ W_out.T + b_out;  x_{t+1} = hn
  out = (B, 128, 64)

Because x_{t+1} == h_{t+1} for t >= 1, the two GRU matmuls fuse into one
(B,R) @ (R,4R) matmul with W_fused rows [Wi_r+Wh_r; Wi_z+Wh_z; Wi_n; Wh_n].

Sharding: data-parallel over batch, B=2048 -> 8 cores x 256. All weights
replicated. On-chip layout is transposed (feature dim on partitions, batch on
the free dim) so gate biases are per-partition scalars and the recurrent
matmuls keep weights stationary:
    gatesT[4R, b] = W_fused @ hT   via  matmul(out, lhsT=W_fused.T, rhs=hT)

Engine split per step (chunk c = feature rows c*128..c*128+127):
  PE:   16 gate MMs (N=256 f16) + logits (2 MMs + K=1 ones-MM injecting
        b_out into PSUM, so logits DMA straight from PSUM with no ACT pass)
        + one K=2 selector bias-MM injecting b_z into the gz bank (so the
        u-sigmoid is a single [128,512] ACT instr).
  ACT:  r = sigmoid per chunk (bias in ACT), u = sigmoid [128,512],
        n = tanh per chunk (bias pre-added by the pre-STT).
  DVE:  t2_c=(b_hn+ghn)*r (STT), pre_c=(b_in+gin)+t2 (STT),
        p1n_c=(u-1)*n (STT f16), h'_0=p2_0-p1n_0 (chain chunk).
  Pool: p2_c=u*h, h'_1 (off-chain chunk, staggered for the k1 MMs).

All matmul operands f16 (measured end-to-end rel err ~1e-3 scale); gate
accumulation and logits stay f32 in PSUM.
"""

import numpy as np

import concourse.bass as bass
import concourse.tile as tile
from concourse import bacc, mybir
from concourse.bass_utils import run_bass_kernel_spmd

F32 = mybir.dt.float32
F16 = mybir.dt.float16
ACT = mybir.ActivationFunctionType
ALU = mybir.AluOpType

B = 2048
HID = 256
COND = 128
NCH = 64
MAXLEN = 128
R = 256
NCORES = 8
BC = B // NCORES  # 256 per-core batch
KT = R // 128     # 2 k-tiles over R
ZC = HID + COND   # 384
ZKT = ZC // 128   # 3 k-tiles over hid+cond


def _build(maxlen=MAXLEN, dbg=False):
    nc = bacc.Bacc("TRN2", target_bir_lowering=False, debug=False)

    # ---- DRAM I/O (per-core shapes) ----
    d_zct = nc.dram_tensor("zct", [ZC, BC], F16, kind="ExternalInput")
    d_wf = nc.dram_tensor("wft", [R, 4 * R], F16, kind="ExternalInput")
    d_whh = nc.dram_tensor("whht", [R, 3 * R], F16, kind="ExternalInput")
    d_wlh = nc.dram_tensor("wlht", [ZC, R], F16, kind="ExternalInput")
    d_wout = nc.dram_tensor("woutt", [R, NCH], F16, kind="ExternalInput")
    # per-partition bias columns: 0,1=b_r  2,3=b_hn  4,5=b_in (per chunk)
    d_bias = nc.dram_tensor("biases", [128, 6], F32, kind="ExternalInput")
    # selector-MM operands
    d_bzl = nc.dram_tensor("bzl", [KT, 128], F16, kind="ExternalInput")
    d_blhl = nc.dram_tensor("blhl", [KT, 128], F16, kind="ExternalInput")
    d_boutl = nc.dram_tensor("boutl", [1, NCH], F16, kind="ExternalInput")
    d_binl = nc.dram_tensor("binl", [1, KT * 128], F16, kind="ExternalInput")
    d_ident = nc.dram_tensor("ident", [128, 128], F16, kind="ExternalInput")
    d_ones = nc.dram_tensor("ones1", [1, BC], F16, kind="ExternalInput")
    d_sel2 = nc.dram_tensor("sel2", [KT, KT * BC], F16, kind="ExternalInput")
    d_out = nc.dram_tensor("out", [maxlen, NCH, BC], F32, kind="ExternalOutput")
    d_ka = nc.dram_tensor("kadbg", [128, BC], F32, kind="ExternalOutput")
    d_dbg = {}
    if dbg:
        for nm in ("h0", "r", "u", "t2", "nt", "p2", "p1n", "hs", "gr",
                   "ghn", "gin"):
            d_dbg[nm] = nc.dram_tensor("dbg_" + nm, [128, KT, BC], F32,
                                       kind="ExternalOutput")

    with tile.TileContext(nc) as tc:
        with (
            tc.tile_pool(name="const", bufs=1) as const,
            tc.tile_pool(name="state", bufs=1) as state,
            tc.tile_pool(name="ew", bufs=2) as ew,
            tc.tile_pool(name="pgr", bufs=1, space="PSUM") as pgr,
            tc.tile_pool(name="pghn", bufs=1, space="PSUM") as pghn,
            tc.tile_pool(name="pgin", bufs=1, space="PSUM") as pgin,
            tc.tile_pool(name="pgz", bufs=1, space="PSUM") as pgz,
            tc.tile_pool(name="pl", bufs=1, space="PSUM") as pl,
            tc.tile_pool(name="ph0", bufs=1, space="PSUM") as ph0,
        ):
            # ---- load constants ----
            wf = const.tile([128, KT, 4 * R], F16)
            nc.sync.dma_start(wf, d_wf[:].rearrange("(k p) m -> p k m", p=128))
            whh = const.tile([128, KT, 3 * R], F16)
            nc.sync.dma_start(whh, d_whh[:].rearrange("(k p) m -> p k m", p=128))
            wlh = const.tile([128, ZKT, R], F16)
            nc.sync.dma_start(wlh, d_wlh[:].rearrange("(k p) m -> p k m", p=128))
            wout = const.tile([128, KT, NCH], F16)
            nc.sync.dma_start(wout, d_wout[:].rearrange("(k p) m -> p k m", p=128))
            zct = const.tile([128, ZKT, BC], F16)
            nc.sync.dma_start(zct, d_zct[:].rearrange("(k p) m -> p k m", p=128))
            bia = const.tile([128, 6], F32)
            nc.sync.dma_start(bia, d_bias[:])
            bzl = const.tile([KT, 128], F16)
            nc.sync.dma_start(bzl, d_bzl[:])
            blhl = const.tile([KT, 128], F16)
            nc.sync.dma_start(blhl, d_blhl[:])
            boutr1 = const.tile([1, NCH], F16)
            nc.sync.dma_start(boutr1, d_boutl[:])
            binl = const.tile([1, KT * 128], F16)
            nc.sync.dma_start(binl, d_binl[:])
            ident = const.tile([128, 128], F16)
            nc.sync.dma_start(ident, d_ident[:])
            ones1 = const.tile([1, BC], F16)
            nc.sync.dma_start(ones1, d_ones[:])
            sel2 = const.tile([KT, KT * BC], F16)
            nc.sync.dma_start(sel2, d_sel2[:])

            def bcol(i):
                return bia[:, i : i + 1]

            h = state.tile([128, KT, BC], F16)  # hT, chunk c = rows c*128..

            # PE keepalive: dummy accumulating matmuls on chain tiles keep
            # the HAM clock-gate at full rate through the per-step EW gap.
            ka = ph0.tile([128, KT, BC], F32, tag="ka")
            ka_started = [False]

            def keepalive_on(rhs):
                n = rhs.free_size()
                nc.tensor.matmul(ka[:, 0, 0:n] if n <= BC else ka[:],
                                 wf[:, 0, 0:128], rhs,
                                 start=(not ka_started[0]), stop=False,
                                 skip_group_check=True)
                ka_started[0] = True

            # ---- HAM warmup: ~4.5us of back-to-back MMs so the SHORT
            # window fires and the PE clock un-throttles to 2.4 GHz; the
            # per-step keepalives then keep every MID window non-idle.
            for _ in range(11):
                keepalive_on(zct[:, 0:2, :])

            # ---- h0 = W_lh @ zcT + b_lh (selector bias-MM + 3 k-tiles) ----
            # (borrows the gz pool's bank; freed before step 0 writes gz)
            ph = pgz.tile([128, KT, BC], F32, tag="gz")
            nc.tensor.matmul(ph[:], blhl[:], sel2[:],
                             start=True, stop=False, skip_group_check=True)
            for m in range(KT):
                for k in range(ZKT):
                    nc.tensor.matmul(ph[:, m, :], wlh[:, k, bass.ts(m, 128)],
                                     zct[:, k, :], start=False,
                                     stop=(m == KT - 1 and k == ZKT - 1),
                                     skip_group_check=True)
            nc.scalar.activation(h[:],
                                 ph[:], ACT.Identity,
                                 bias=0.0)
            dbg_f32 = {}
            if dbg:
                for nm in d_dbg:
                    dbg_f32[nm] = const.tile([128, KT, BC], F32,
                                             name="dbgt_" + nm)
                nc.vector.tensor_copy(dbg_f32["h0"], h[:])
                nc.sync.dma_start(d_dbg["h0"][:], dbg_f32["h0"])

            # pending logits emitters from the previous step
            pending = []
            pending_io = []

            def flush_logits():
                for fn in pending:
                    fn()
                pending.clear()

            def flush_logits_io():
                for fn in pending_io:
                    fn()
                pending_io.clear()

            def emit_step(t, first):
                if first:
                    # x=0: gates come from W_hh only (pytorch order r,z,n)
                    w, offs = whh, {"r": 0, "z": R, "hn": 2 * R}
                else:
                    w, offs = wf, {"r": 0, "z": R, "in": 2 * R, "hn": 3 * R}

                gr = pgr.tile([128, KT, BC], F32, tag="gr")
                ghn = pghn.tile([128, KT, BC], F32, tag="ghn")
                gin0 = pgin.tile([128, BC], F32, tag="gin0")
                gin1 = pgin.tile([128, BC], F32, tag="gin1")
                ginm = (gin0, gin1)
                gz = pgz.tile([128, KT, BC], F32, tag="gz")

                def gate_mm(dst, name, m, k, start, stop):
                    d = dst[m] if isinstance(dst, tuple) else dst[:, m, :]
                    nc.tensor.matmul(
                        d, w[:, k, bass.ds(offs[name] + m * 128, 128)],
                        h[:, k, :], start=start, stop=stop,
                        skip_group_check=True)

                # ---- PE burst (gate-major: r completes first so the
                # chain-head sigmoids unblock ~700ns earlier) ----
                # gz bias-MM first: no h dependency, runs during prev EW tail
                nc.tensor.matmul(gz[:], bzl[:], sel2[:],
                                 start=True, stop=False, skip_group_check=True)
                # b_in ones-MMs into the gin tiles (gin0 first: its
                # start=True marks the shared bank's zero region)
                for m in range(KT):
                    nc.tensor.matmul(ginm[m], binl[:, bass.ts(m, 128)],
                                     ones1[:],
                                     start=True, stop=False,
                                     skip_group_check=True)
                names = ("r", "hn") if first else ("r", "hn", "in")
                for name in names + ("z",):
                    dst = {"r": gr, "hn": ghn, "in": ginm, "z": gz}[name]
                    for m in range(KT):
                        for k in range(KT):
                            gate_mm(dst, name, m, k,
                                    start=(name in ("r", "hn") and m == 0
                                           and k == 0),
                                    stop=(name != "in" and m == KT - 1
                                          and k == KT - 1))
                # t2 -> gin bank via identity-weight MMs (replaces pre-STT)
                # (emitted after the elementwise t2 below via a closure list)

                if dbg and t == 0:
                    nc.vector.tensor_copy(dbg_f32["gin"][:, 0, :], gin0[:])
                    nc.vector.tensor_copy(dbg_f32["gin"][:, 1, :], gin1[:])
                    nc.sync.dma_start(d_dbg["gin"][:], dbg_f32["gin"])
                    nc.vector.tensor_copy(dbg_f32["gr"], gr[:])
                    nc.sync.dma_start(d_dbg["gr"][:], dbg_f32["gr"])
                    nc.vector.tensor_copy(dbg_f32["ghn"], ghn[:])
                    nc.sync.dma_start(d_dbg["ghn"][:], dbg_f32["ghn"])
                # ---- ACT: r per chunk (chain head), u merged ----
                r = ew.tile([128, KT, BC], F16, tag="r")
                with tc.high_priority(64):
                    for m in range(KT):
                        nc.scalar.activation(r[:, m, :], gr[:, m, :],
                                             ACT.Sigmoid, bias=bcol(0 + m))
                # ---- DVE: t2 = (b_hn + ghn) * r, then identity-MMs fold
                # t2 into the gin tiles (pre-sum in PSUM), tanh reads them.
                # ACT order [r0, r1, tanh0, u, tanh1]: u's 687ns sits in the
                # idMM-1 window instead of blocking tanh-0.
                u = ew.tile([128, KT, BC], F16, tag="u")
                t2 = ew.tile([128, KT, BC], F16, tag="t2")
                nt = ew.tile([128, KT, BC], F16, tag="nt")
                for m in range(KT):
                    nc.vector.scalar_tensor_tensor(
                        t2[:, m, :], ghn[:, m, :], bcol(2 + m), r[:, m, :],
                        op0=ALU.add, op1=ALU.mult)
                    nc.tensor.matmul(ginm[m], ident[:], t2[:, m, :],
                                     start=False, stop=True,
                                     skip_group_check=True)
                    nc.scalar.activation(nt[:, m, :], ginm[m], ACT.Tanh,
                                         bias=0.0)
                    if m == 0:
                        nc.scalar.activation(u[:], gz[:], ACT.Sigmoid,
                                             bias=0.0)

                # pending logits from the previous step: must be emitted
                # before this step's h'-writes (program order fixes which h
                # version the MMs read), but sits late in the PE/ACT queues
                flush_logits()

                # ---- combine: p2 = u*h, p1n = (u-1)*n, h' = p2 - p1n ----
                p1n = ew.tile([128, KT, BC], F16, tag="p1n")
                p2 = ew.tile([128, KT, BC], F16, tag="p2")
                # p2 chunk0 on DVE (fits in the tanh-0 window), chunk1 Pool
                nc.vector.tensor_mul(p2[:, 0, :], u[:, 0, :], h[:, 0, :])
                nc.gpsimd.tensor_mul(p2[:, 1, :], u[:, 1, :], h[:, 1, :])
                with tc.high_priority(64):
                    nc.vector.scalar_tensor_tensor(
                        p1n[:, 0, :], u[:, 0, :], -1.0, nt[:, 0, :],
                        op0=ALU.add, op1=ALU.mult)
                    nc.vector.tensor_sub(h[:, 0, :], p2[:, 0, :],
                                         p1n[:, 0, :])
                nc.vector.scalar_tensor_tensor(
                    p1n[:, 1, :], u[:, 1, :], -1.0, nt[:, 1, :],
                    op0=ALU.add, op1=ALU.mult)
                nc.vector.tensor_sub(h[:, 1, :], p2[:, 1, :], p1n[:, 1, :])
                if dbg and t == 0:
                    for nm, tl in (("r", r), ("u", u), ("t2", t2),
                                   ("nt", nt), ("p2", p2),
                                   ("p1n", p1n), ("hs", h)):
                        nc.vector.tensor_copy(dbg_f32[nm], tl[:])
                        nc.sync.dma_start(d_dbg[nm][:], dbg_f32[nm])

                # keepalives spread through the EW tail (fat N=512 MMs
                # keep the PE duty cycle high enough that HAM stays warm)
                keepalive_on(r[:])
                keepalive_on(t2[:, 0, :])
                keepalive_on(t2[:])
                keepalive_on(t2[:, 1, :])
                keepalive_on(nt[:, 1, :])
                keepalive_on(nt[:, 0, :])
                keepalive_on(nt[:])
                keepalive_on(p1n[:])

                # ---- logits (deferred into next burst) ----
                def do_logits(t=t):
                    lp = pl.tile([NCH, BC], F32, tag="lp")
                    nc.tensor.matmul(lp, boutr1[:], ones1[:],
                                     start=True, stop=False,
                                     skip_group_check=True)
                    for k in range(KT):
                        nc.tensor.matmul(lp, wout[:, k, :], h[:, k, :],
                                         start=False, stop=(k == KT - 1),
                                         skip_group_check=True)

                    def do_io(lp=lp, t=t):
                        ls = ew.tile([NCH, BC], F32, tag="ls")
                        nc.vector.tensor_copy(ls, lp)
                        nc.sync.dma_start(d_out[t], ls)
                    pending_io.append(do_io)
                pending.append(do_logits)

            emit_step(0, first=True)
            for t in range(1, maxlen):
                emit_step(t, first=False)
            flush_logits()
            flush_logits_io()
            kcopy = ew.tile([128, BC], F32, tag="kcopy")
            nc.scalar.activation(kcopy, ka[:, 0, :], ACT.Identity, bias=0.0)
            nc.sync.dma_start(d_ka[:], kcopy)

    nc.compile()
    return nc


_CACHE = {}
_LAST_IN_MAPS = None


def kernel(z, x_cond, W_lh, b_lh, W_ih, W_hh, b_ih, b_hh, W_out, b_out):
    z = np.asarray(z, np.float32)
    x_cond = np.asarray(x_cond, np.float32)
    W_lh = np.asarray(W_lh, np.float32)
    b_lh = np.asarray(b_lh, np.float32)
    W_ih = np.asarray(W_ih, np.float32)
    W_hh = np.asarray(W_hh, np.float32)
    b_ih = np.asarray(b_ih, np.float32)
    b_hh = np.asarray(b_hh, np.float32)
    W_out = np.asarray(W_out, np.float32)
    b_out = np.asarray(b_out, np.float32)

    # fused recurrent weight: rows [Wi_r+Wh_r; Wi_z+Wh_z; Wi_n; Wh_n]
    Wf = np.concatenate(
        [W_ih[:R] + W_hh[:R], W_ih[R : 2 * R] + W_hh[R : 2 * R],
         W_ih[2 * R :], W_hh[2 * R :]], axis=0)
    b_r = b_ih[:R] + b_hh[:R]
    b_z = b_ih[R : 2 * R] + b_hh[R : 2 * R]
    b_in = b_ih[2 * R :]
    b_hn = b_hh[2 * R :]

    def pcols(v):  # (R,) -> (128, KT) per-partition columns
        return np.ascontiguousarray(v.reshape(KT, 128).T)

    biases = np.ascontiguousarray(
        np.concatenate([pcols(b_r), pcols(b_hn), pcols(b_in)],
                       axis=1))  # (128, 6)

    f16 = np.float16
    wft = np.ascontiguousarray(Wf.T, dtype=f16)            # (R, 4R)
    whht = np.ascontiguousarray(W_hh.T, dtype=f16)         # (R, 3R)
    wlht = np.ascontiguousarray(W_lh.T, dtype=f16)         # (ZC, R)
    woutt = np.ascontiguousarray(W_out.T, dtype=f16)       # (R, NCH)
    bzl = np.ascontiguousarray(b_z.reshape(KT, 128), dtype=f16)
    blhl = np.ascontiguousarray(b_lh.reshape(KT, 128), dtype=f16)
    boutr = np.ascontiguousarray(b_out.reshape(1, NCH), dtype=f16)
    binl = np.ascontiguousarray(b_in.reshape(1, KT * 128), dtype=f16)
    identm = np.ascontiguousarray(np.eye(128), dtype=f16)
    ones1 = np.ones((1, BC), dtype=f16)
    sel2 = np.zeros((KT, KT * BC), dtype=f16)
    for c in range(KT):
        sel2[c, c * BC : (c + 1) * BC] = 1.0
    zct_full = np.concatenate([z, x_cond], axis=1).T.astype(f16)  # (ZC, B)

    if "nc" not in _CACHE:
        _CACHE["nc"] = _build()
    nc = _CACHE["nc"]

    in_maps = []
    for c in range(NCORES):
        in_maps.append({
            "zct": np.ascontiguousarray(zct_full[:, c * BC : (c + 1) * BC]),
            "wft": wft,
            "whht": whht,
            "wlht": wlht,
            "woutt": woutt,
            "biases": biases,
            "bzl": bzl,
            "blhl": blhl,
            "boutl": boutr,
            "binl": binl,
            "ident": identm,
            "ones1": ones1,
            "sel2": sel2,
        })

    global _LAST_IN_MAPS
    _LAST_IN_MAPS = in_maps
    res = run_bass_kernel_spmd(nc, in_maps, core_ids=list(range(NCORES)))
    # per-core out: (MAXLEN, NCH, BC) -> (BC, MAXLEN, NCH)
    parts = [np.asarray(res.results[c]["out"]).transpose(2, 0, 1)
             for c in range(NCORES)]
    return np.ascontiguousarray(np.concatenate(parts, axis=0), dtype=np.float32)
